# revision 2
# baseline (speedup 1.0000x reference)
"""Trainium2 Bass kernel for the nn_AaD retrieval-KNN loss.

Self-contained: takes the FULL unsharded inputs, shards fea_bank/score_bank
row-wise across 8 NeuronCores, runs one SPMD Bass program (distance matmul +
hardware top-8 + score gather + per-candidate KL contributions + dispersion),
then merges the per-core candidates on the host into the global top-5 and the
final scalar loss.
"""

import numpy as np

import concourse.bass as bass
import concourse.mybir as mybir
import concourse.tile as tile
from concourse import bacc
from concourse.bass import IndirectOffsetOnAxis
from concourse.bass_utils import run_bass_kernel_spmd

B, D, C, N, K = 256, 512, 345, 50000, 5
ALPHA = 1.0
EPS = 1e-12
M = 8                 # cores
NS = N // M           # 6250 bank rows per core
G = 13                # column groups per core
GW = 512              # group width (matmul free dim)
NPAD = G * GW         # 6656
LASTW = NS - (G - 1) * GW  # 106 real columns in the last group

F32 = mybir.dt.float32
U32 = mybir.dt.uint32
AF = mybir.ActivationFunctionType
ALU = mybir.AluOpType

_CACHE: dict = {}


def _build():
    nc = bacc.Bacc("TRN2", target_bir_lowering=False, debug=False, num_devices=M)

    fbt_in = nc.dram_tensor("fbt", [G, 4, 128, GW], F32, kind="ExternalInput")
    fnt_in = nc.dram_tensor("fnt", [4, 128, B], F32, kind="ExternalInput")
    sbk_in = nc.dram_tensor("sbk", [NS, C], F32, kind="ExternalInput")
    p_in = nc.dram_tensor("p", [2, 128, C], F32, kind="ExternalInput")

    out_vals = nc.dram_tensor("out_vals", [2, 128, 8], F32, kind="ExternalOutput")
    out_g = nc.dram_tensor("out_g", [2, 128, 8], F32, kind="ExternalOutput")
    out_idx = nc.dram_tensor("out_idx", [2, 128, 8], U32, kind="ExternalOutput")
    out_disp = nc.dram_tensor("out_disp", [1, 1], F32, kind="ExternalOutput")

    with tile.TileContext(nc) as tc:
        with (
            tc.tile_pool(name="const", bufs=1) as constp,
            tc.tile_pool(name="fbgp", bufs=3) as fbgp,
            tc.tile_pool(name="mid", bufs=1) as midp,
            tc.tile_pool(name="scr", bufs=2) as scrp,
            tc.tile_pool(name="psum", bufs=4, space="PSUM") as psp,
            tc.tile_pool(name="psum2", bufs=1, space="PSUM") as psp2,
        ):
            fnt_sb = constp.tile([128, 4 * B], F32, tag="fnt")
            for dk in range(4):
                nc.sync.dma_start(fnt_sb[:, dk * B:(dk + 1) * B], fnt_in[dk])
            p_sb = constp.tile([128, 2 * C], F32, tag="psb")
            for m in range(2):
                nc.sync.dma_start(p_sb[:, m * C:(m + 1) * C], p_in[m])

            dists = [constp.tile([128, NS], F32, tag=f"dist{m}", name=f"dist{m}") for m in range(2)]

            # distance = f_norm @ fb_slab.T, computed as 512-wide column groups
            for g in range(G):
                fbg = fbgp.tile([128, 4 * GW], F32, tag="fbg")
                for dk in range(4):
                    nc.sync.dma_start(fbg[:, dk * GW:(dk + 1) * GW], fbt_in[g, dk])
                w = GW if g < G - 1 else LASTW
                for m in range(2):
                    ps = psp.tile([128, GW], F32, tag="ps")
                    for dk in range(4):
                        nc.tensor.matmul(
                            ps[:],
                            lhsT=fnt_sb[:, dk * B + m * 128: dk * B + m * 128 + 128],
                            rhs=fbg[:, dk * GW:(dk + 1) * GW],
                            start=(dk == 0),
                            stop=(dk == 3),
                        )
                    nc.scalar.activation(dists[m][:, g * GW:g * GW + w], ps[:, :w], AF.Copy)

            # per-core top-8 + gather + per-candidate KL contribution
            for m in range(2):
                mx = midp.tile([128, 8], F32, tag=f"mx{m}")
                ix = midp.tile([128, 8], U32, tag=f"ix{m}")
                nc.vector.max(out=mx[:], in_=dists[m][:])
                nc.vector.max_index(out=ix[:], in_max=mx[:], in_values=dists[m][:])
                nc.sync.dma_start(out_vals[m], mx[:])
                nc.sync.dma_start(out_idx[m], ix[:])

                sbg = midp.tile([128, 8 * C], F32, tag=f"sbg{m}")
                for j in range(8):
                    nc.gpsimd.indirect_dma_start(
                        out=sbg[:, j * C:(j + 1) * C],
                        out_offset=None,
                        in_=sbk_in[:, :],
                        in_offset=IndirectOffsetOnAxis(ap=ix[:, j:j + 1], axis=0),
                    )

                tln = midp.tile([128, 8 * C], F32, tag=f"tln{m}")
                nc.scalar.activation(tln[:], sbg[:], AF.Ln)

                # H_j = sum_c sb*ln(sb); q_j = sum_c sb*p;  g_j = H_j - q_j
                hq = midp.tile([128, 16], F32, tag=f"hq{m}")
                for j in range(8):
                    scr = scrp.tile([128, C], F32, tag="scr")
                    nc.vector.scalar_tensor_tensor(
                        out=scr[:],
                        in0=tln[:, j * C:(j + 1) * C],
                        scalar=0.0,
                        in1=sbg[:, j * C:(j + 1) * C],
                        op0=ALU.add,
                        op1=ALU.mult,
                        accum_out=hq[:, j:j + 1],
                    )
                    scr2 = scrp.tile([128, C], F32, tag="scr")
                    nc.vector.scalar_tensor_tensor(
                        out=scr2[:],
                        in0=p_sb[:, m * C:(m + 1) * C],
                        scalar=0.0,
                        in1=sbg[:, j * C:(j + 1) * C],
                        op0=ALU.add,
                        op1=ALU.mult,
                        accum_out=hq[:, 8 + j:9 + j],
                    )
                g8 = midp.tile([128, 8], F32, tag=f"g8{m}")
                nc.vector.tensor_tensor(out=g8[:], in0=hq[:, 0:8], in1=hq[:, 8:16], op=ALU.subtract)
                nc.sync.dma_start(out_g[m], g8[:])

            # dispersion: (||sum_b p_b||^2 - sum_b ||p_b||^2) / B
            ones = constp.tile([128, 1], F32, tag="ones")
            nc.vector.memset(ones[:], 1.0)
            rowsq = constp.tile([128, 2], F32, tag="rowsq")
            for m in range(2):
                scr = scrp.tile([128, C], F32, tag="scr")
                nc.vector.scalar_tensor_tensor(
                    out=scr[:],
                    in0=p_sb[:, m * C:(m + 1) * C],
                    scalar=0.0,
                    in1=p_sb[:, m * C:(m + 1) * C],
                    op0=ALU.add,
                    op1=ALU.mult,
                    accum_out=rowsq[:, m:m + 1],
                )
            s_ps = psp2.tile([1, C], F32, tag="S")
            for m in range(2):
                nc.tensor.matmul(
                    s_ps[:], lhsT=ones[:], rhs=p_sb[:, m * C:(m + 1) * C],
                    start=(m == 0), stop=(m == 1),
                )
            rq_ps = psp2.tile([1, 2], F32, tag="rq")
            nc.tensor.matmul(rq_ps[:], lhsT=ones[:], rhs=rowsq[:], start=True, stop=True)

            s_sb = constp.tile([1, C], F32, tag="ssb")
            nc.scalar.activation(s_sb[:], s_ps[:], AF.Copy)
            rq_sb = constp.tile([1, 2], F32, tag="rqsb")
            nc.scalar.activation(rq_sb[:], rq_ps[:], AF.Copy)
            scr_s = constp.tile([1, C], F32, tag="scrS")
            ssq = constp.tile([1, 1], F32, tag="ssq")
            nc.vector.scalar_tensor_tensor(
                out=scr_s[:], in0=s_sb[:], scalar=0.0, in1=s_sb[:],
                op0=ALU.add, op1=ALU.mult, accum_out=ssq[:],
            )
            t1 = constp.tile([1, 1], F32, tag="t1")
            nc.vector.tensor_tensor(out=t1[:], in0=rq_sb[:, 0:1], in1=rq_sb[:, 1:2], op=ALU.add)
            t2 = constp.tile([1, 1], F32, tag="t2")
            nc.vector.tensor_tensor(out=t2[:], in0=ssq[:], in1=t1[:], op=ALU.subtract)
            t3 = constp.tile([1, 1], F32, tag="t3")
            nc.vector.tensor_scalar_mul(t3[:], t2[:], 1.0 / B)
            nc.sync.dma_start(out_disp[:], t3[:])

    nc.compile()
    return nc


def _get_nc():
    if "nc" not in _CACHE:
        _CACHE["nc"] = _build()
    return _CACHE["nc"]


def _prep(features, predictions, fea_bank, score_bank, trg_idx):
    feat = np.asarray(features, dtype=np.float32)
    pred = np.asarray(predictions, dtype=np.float32)
    fb = np.array(fea_bank, dtype=np.float32)
    sb = np.array(score_bank, dtype=np.float32)
    trg = np.asarray(trg_idx).astype(np.int64)

    x = pred - pred.max(axis=1, keepdims=True)
    e = np.exp(x)
    p = e / e.sum(axis=1, keepdims=True)

    nrm = np.sqrt((feat * feat).sum(axis=1, keepdims=True))
    fn = feat / np.maximum(nrm, EPS)

    fb[trg] = fn
    sb[trg] = p

    fnt = np.ascontiguousarray(fn.T.reshape(4, 128, B))
    p_dev = np.ascontiguousarray(p.reshape(2, 128, C))

    in_maps = []
    for c in range(M):
        slab = fb[c * NS:(c + 1) * NS]
        fbt = np.zeros((D, NPAD), dtype=np.float32)
        fbt[:, :NS] = slab.T
        fbt = np.ascontiguousarray(fbt.reshape(4, 128, G, GW).transpose(2, 0, 1, 3))
        sbk = np.ascontiguousarray(sb[c * NS:(c + 1) * NS])
        in_maps.append({"fbt": fbt, "fnt": fnt, "sbk": sbk, "p": p_dev})
    return in_maps


def _merge(results):
    vals, gs, gidx = [], [], []
    for c in range(M):
        r = results[c]
        vals.append(r["out_vals"].reshape(B, 8))
        gs.append(r["out_g"].reshape(B, 8))
        gidx.append(r["out_idx"].reshape(B, 8).astype(np.int64) + c * NS)
    v = np.concatenate(vals, axis=1)
    g = np.concatenate(gs, axis=1)
    gi = np.concatenate(gidx, axis=1)

    # global top-(K+1) by value, ties -> lowest original index (lax.top_k)
    order = np.lexsort((gi, -v.astype(np.float64)), axis=-1)
    sel = order[:, 1:K + 1]  # drop rank 0
    kl = np.take_along_axis(g, sel, axis=1).astype(np.float64).sum(axis=1).mean()
    disp = float(results[0]["out_disp"][0, 0])
    return np.float32(kl + ALPHA * disp)


def run(inputs, trace=False):
    nc = _get_nc()
    in_maps = _prep(**inputs)
    res = run_bass_kernel_spmd(nc, in_maps, list(range(M)), trace=trace)
    return _merge(res.results), res


def kernel(features, predictions, fea_bank, score_bank, trg_idx):
    loss, _ = run(
        dict(
            features=features,
            predictions=predictions,
            fea_bank=fea_bank,
            score_bank=score_bank,
            trg_idx=trg_idx,
        )
    )
    return loss


# revision 4
# speedup vs baseline: 1.4494x; 1.4494x over previous
"""Trainium2 Bass kernel for the nn_AaD retrieval-KNN loss.

Self-contained: takes the FULL unsharded inputs, shards fea_bank/score_bank
row-wise across 8 NeuronCores, runs one SPMD Bass program (distance matmul +
hardware top-8 + score gather + per-candidate KL contributions + dispersion),
then merges the per-core candidates on the host into the global top-5 and the
final scalar loss.
"""

import numpy as np

import concourse.bass as bass
import concourse.mybir as mybir
import concourse.tile as tile
from concourse import bacc
from concourse.bass import IndirectOffsetOnAxis
from concourse.bass_utils import run_bass_kernel_spmd

B, D, C, N, K = 256, 512, 345, 50000, 5
ALPHA = 1.0
EPS = 1e-12
M = 8                 # cores
NS = N // M           # 6250 bank rows per core
G = 13                # column groups per core
GW = 512              # group width (matmul free dim)
NPAD = G * GW         # 6656
LASTW = NS - (G - 1) * GW  # 106 real columns in the last group

F32 = mybir.dt.float32
F32R = mybir.dt.float32r
U32 = mybir.dt.uint32
AF = mybir.ActivationFunctionType
ALU = mybir.AluOpType

_CACHE: dict = {}


def _build():
    nc = bacc.Bacc("TRN2", target_bir_lowering=False, debug=False, num_devices=M)

    fbt_in = nc.dram_tensor("fbt", [G, 4, 128, GW], F32R, kind="ExternalInput")
    fnt_in = nc.dram_tensor("fnt", [4, 128, B], F32R, kind="ExternalInput")
    sbk_in = nc.dram_tensor("sbk", [NS, C], F32, kind="ExternalInput")
    p_in = nc.dram_tensor("p", [2, 128, C], F32, kind="ExternalInput")

    out_vals = nc.dram_tensor("out_vals", [2, 128, 8], F32, kind="ExternalOutput")
    out_g = nc.dram_tensor("out_g", [2, 128, 8], F32, kind="ExternalOutput")
    out_idx = nc.dram_tensor("out_idx", [2, 128, 8], U32, kind="ExternalOutput")
    out_disp = nc.dram_tensor("out_disp", [1, 1], F32, kind="ExternalOutput")

    with tile.TileContext(nc) as tc:
        with (
            tc.tile_pool(name="const", bufs=1) as constp,
            tc.tile_pool(name="fbgp", bufs=3) as fbgp,
            tc.tile_pool(name="mid", bufs=1) as midp,
            tc.tile_pool(name="scr", bufs=2) as scrp,
            tc.tile_pool(name="psum", bufs=4, space="PSUM") as psp,
            tc.tile_pool(name="psum2", bufs=1, space="PSUM") as psp2,
        ):
            fnt_sb = constp.tile([128, 4 * B], F32R, tag="fnt")
            nc.sync.dma_start(fnt_sb[:].rearrange("p (a b) -> p a b", a=4), fnt_in[:].rearrange("a p b -> p a b"))
            p_sb = constp.tile([128, 2 * C], F32, tag="psb")
            nc.sync.dma_start(p_sb[:].rearrange("p (m c) -> p m c", m=2), p_in[:].rearrange("m p c -> p m c"))

            dists = [constp.tile([128, NS], F32, tag=f"dist{m}", name=f"dist{m}") for m in range(2)]

            # distance = f_norm @ fb_slab.T, computed as 512-wide column groups.
            # Groups are processed in pairs so each LDWEIGHTS serves two matmuls.
            fbgs = {}
            for g in range(G):
                fbg = fbgp.tile([128, 4 * GW], F32R, tag="fbg", name=f"fbg{g % 4}")
                nc.sync.dma_start(fbg[:].rearrange("p (a b) -> p a b", a=4), fbt_in[g].rearrange("a p b -> p a b"))
                fbgs[g] = fbg
            for g0 in range(0, G, 2):
                pair = [g for g in (g0, g0 + 1) if g < G]
                for m in range(2):
                    pss = {}
                    for g in pair:
                        pss[g] = psp.tile([128, GW], F32, tag="ps", name=f"ps{g % 2}_{m}")
                    for dk in range(4):
                        for g in pair:
                            nc.tensor.matmul(
                                pss[g][:],
                                lhsT=fnt_sb[:, dk * B + m * 128: dk * B + m * 128 + 128],
                                rhs=fbgs[g][:, dk * GW:(dk + 1) * GW],
                                start=(dk == 0),
                                stop=(dk == 3),
                            )
                    for g in pair:
                        w = GW if g < G - 1 else LASTW
                        nc.scalar.activation(dists[m][:, g * GW:g * GW + w], pss[g][:, :w], AF.Copy)

            # per-core top-8 + gather + per-candidate KL contribution
            for m in range(2):
                mx = midp.tile([128, 8], F32, tag=f"mx{m}")
                ix = midp.tile([128, 8], U32, tag=f"ix{m}")
                nc.vector.max(out=mx[:], in_=dists[m][:])
                nc.vector.max_index(out=ix[:], in_max=mx[:], in_values=dists[m][:])
                nc.sync.dma_start(out_vals[m], mx[:])
                nc.sync.dma_start(out_idx[m], ix[:])

                sbg = midp.tile([128, 8 * C], F32, tag=f"sbg{m}")
                nc.gpsimd.indirect_dma_start(
                    out=sbg[:],
                    out_offset=None,
                    in_=sbk_in[:, :],
                    in_offset=IndirectOffsetOnAxis(ap=ix[:, 0:8], axis=0),
                )

                tln = midp.tile([128, 8 * C], F32, tag=f"tln{m}")
                nc.scalar.activation(tln[:], sbg[:], AF.Ln)

                # H_j = sum_c sb*ln(sb); q_j = sum_c sb*p;  g_j = H_j - q_j
                hq = midp.tile([128, 16], F32, tag=f"hq{m}")
                for j in range(8):
                    scr = scrp.tile([128, C], F32, tag="scr")
                    nc.vector.scalar_tensor_tensor(
                        out=scr[:],
                        in0=tln[:, j * C:(j + 1) * C],
                        scalar=0.0,
                        in1=sbg[:, j * C:(j + 1) * C],
                        op0=ALU.add,
                        op1=ALU.mult,
                        accum_out=hq[:, j:j + 1],
                    )
                    scr2 = scrp.tile([128, C], F32, tag="scr")
                    nc.vector.scalar_tensor_tensor(
                        out=scr2[:],
                        in0=p_sb[:, m * C:(m + 1) * C],
                        scalar=0.0,
                        in1=sbg[:, j * C:(j + 1) * C],
                        op0=ALU.add,
                        op1=ALU.mult,
                        accum_out=hq[:, 8 + j:9 + j],
                    )
                g8 = midp.tile([128, 8], F32, tag=f"g8{m}")
                nc.vector.tensor_tensor(out=g8[:], in0=hq[:, 0:8], in1=hq[:, 8:16], op=ALU.subtract)
                nc.sync.dma_start(out_g[m], g8[:])

            # dispersion: (||sum_b p_b||^2 - sum_b ||p_b||^2) / B
            ones = constp.tile([128, 1], F32, tag="ones")
            nc.vector.memset(ones[:], 1.0)
            rowsq = constp.tile([128, 2], F32, tag="rowsq")
            for m in range(2):
                scr = scrp.tile([128, C], F32, tag="scr")
                nc.vector.scalar_tensor_tensor(
                    out=scr[:],
                    in0=p_sb[:, m * C:(m + 1) * C],
                    scalar=0.0,
                    in1=p_sb[:, m * C:(m + 1) * C],
                    op0=ALU.add,
                    op1=ALU.mult,
                    accum_out=rowsq[:, m:m + 1],
                )
            s_ps = psp2.tile([1, C], F32, tag="S")
            for m in range(2):
                nc.tensor.matmul(
                    s_ps[:], lhsT=ones[:], rhs=p_sb[:, m * C:(m + 1) * C],
                    start=(m == 0), stop=(m == 1),
                )
            rq_ps = psp2.tile([1, 2], F32, tag="rq")
            nc.tensor.matmul(rq_ps[:], lhsT=ones[:], rhs=rowsq[:], start=True, stop=True)

            s_sb = constp.tile([1, C], F32, tag="ssb")
            nc.scalar.activation(s_sb[:], s_ps[:], AF.Copy)
            rq_sb = constp.tile([1, 2], F32, tag="rqsb")
            nc.scalar.activation(rq_sb[:], rq_ps[:], AF.Copy)
            scr_s = constp.tile([1, C], F32, tag="scrS")
            ssq = constp.tile([1, 1], F32, tag="ssq")
            nc.vector.scalar_tensor_tensor(
                out=scr_s[:], in0=s_sb[:], scalar=0.0, in1=s_sb[:],
                op0=ALU.add, op1=ALU.mult, accum_out=ssq[:],
            )
            t1 = constp.tile([1, 1], F32, tag="t1")
            nc.vector.tensor_tensor(out=t1[:], in0=rq_sb[:, 0:1], in1=rq_sb[:, 1:2], op=ALU.add)
            t2 = constp.tile([1, 1], F32, tag="t2")
            nc.vector.tensor_tensor(out=t2[:], in0=ssq[:], in1=t1[:], op=ALU.subtract)
            t3 = constp.tile([1, 1], F32, tag="t3")
            nc.vector.tensor_scalar_mul(t3[:], t2[:], 1.0 / B)
            nc.sync.dma_start(out_disp[:], t3[:])

    nc.compile()
    return nc


def _get_nc():
    if "nc" not in _CACHE:
        _CACHE["nc"] = _build()
    return _CACHE["nc"]


def _prep(features, predictions, fea_bank, score_bank, trg_idx):
    feat = np.asarray(features, dtype=np.float32)
    pred = np.asarray(predictions, dtype=np.float32)
    fb = np.array(fea_bank, dtype=np.float32)
    sb = np.array(score_bank, dtype=np.float32)
    trg = np.asarray(trg_idx).astype(np.int64)

    x = pred - pred.max(axis=1, keepdims=True)
    e = np.exp(x)
    p = e / e.sum(axis=1, keepdims=True)

    nrm = np.sqrt((feat * feat).sum(axis=1, keepdims=True))
    fn = feat / np.maximum(nrm, EPS)

    fb[trg] = fn
    sb[trg] = p

    fnt = np.ascontiguousarray(fn.T.reshape(4, 128, B))
    p_dev = np.ascontiguousarray(p.reshape(2, 128, C))

    in_maps = []
    for c in range(M):
        slab = fb[c * NS:(c + 1) * NS]
        fbt = np.zeros((D, NPAD), dtype=np.float32)
        fbt[:, :NS] = slab.T
        fbt = np.ascontiguousarray(fbt.reshape(4, 128, G, GW).transpose(2, 0, 1, 3))
        sbk = np.ascontiguousarray(sb[c * NS:(c + 1) * NS])
        in_maps.append({"fbt": fbt, "fnt": fnt, "sbk": sbk, "p": p_dev})
    return in_maps


def _merge(results):
    vals, gs, gidx = [], [], []
    for c in range(M):
        r = results[c]
        vals.append(r["out_vals"].reshape(B, 8))
        gs.append(r["out_g"].reshape(B, 8))
        gidx.append(r["out_idx"].reshape(B, 8).astype(np.int64) + c * NS)
    v = np.concatenate(vals, axis=1)
    g = np.concatenate(gs, axis=1)
    gi = np.concatenate(gidx, axis=1)

    # global top-(K+1) by value, ties -> lowest original index (lax.top_k)
    order = np.lexsort((gi, -v.astype(np.float64)), axis=-1)
    sel = order[:, 1:K + 1]  # drop rank 0
    kl = np.take_along_axis(g, sel, axis=1).astype(np.float64).sum(axis=1).mean()
    disp = float(results[0]["out_disp"][0, 0])
    return np.float32(kl + ALPHA * disp)


def run(inputs, trace=False):
    nc = _get_nc()
    in_maps = _prep(**inputs)
    res = run_bass_kernel_spmd(nc, in_maps, list(range(M)), trace=trace)
    return _merge(res.results), res


def kernel(features, predictions, fea_bank, score_bank, trg_idx):
    loss, _ = run(
        dict(
            features=features,
            predictions=predictions,
            fea_bank=fea_bank,
            score_bank=score_bank,
            trg_idx=trg_idx,
        )
    )
    return loss
